# Initial kernel scaffold
#
"""Trainium2 Bass kernel: memory-slot cross-attention (nn_LocalConstructorMulti).

Reference computation (per batch b):
    Q  = memory_slots @ Wq.T                      [slots, BD]    (shared over b)
    K  = hs_b @ Wk.T                              [S, BD]
    V  = hs_b @ Wv.T                              [S, BD]
    s  = (Q_h . K_h) / sqrt(HD)  + mask           [heads, slots, S]
    p  = softmax(s, axis=S)
    o  = p @ V_h                                  [heads, slots, HD]
    y  = concat_h(o) @ Wo.T                       [slots, HID]

Sharding: 8 cores = 4 batches x 2 head-groups (4 heads / 256 bottleneck dims
each).  Each core sees the full (transposed, bf16) hidden states of its batch
and a 256-wide slice of Wq/Wk/Wv/Wo, computes the full softmax locally over
its heads, and produces a partial y (contribution of its 4 heads).  The host
sums the two partials per batch -- o_proj is linear in the per-head outputs,
so no flash-softmax combine is needed.

Device layout notes:
  - hs arrives pre-transposed as hsT [HID, S] so the contraction dim (HID) is
    on partitions for both the K-path (hs as moving operand) and the V-path
    (hs as stationary operand).  No on-device transposes of the big tensor.
  - K is built as KT [256, S] (bd on partitions) for the Q.K matmuls;
    V is built as V [S, 256] (rows on partitions) for the p@V matmuls.
  - scores are built transposed, sT [rows, heads*slots], so the additive
    sequence mask is a per-partition bias fused into the Exp activation.
  - softmax denominator comes for free: V tiles carry an extra ones column,
    so o_psum[:, 64] accumulates sum(p) and normalization is a per-partition
    tensor_scalar multiply.
"""

import sys

if "/opt/trn_rl_repo" not in sys.path:
    sys.path.insert(0, "/opt/trn_rl_repo")

import ml_dtypes
import numpy as np

import concourse.bass as bass  # noqa: F401  (AP helpers)
import concourse.mybir as mybir
import concourse.tile as tile
from concourse import bacc
from concourse.bass_utils import run_bass_kernel_spmd
from concourse.masks import make_identity

BF16 = mybir.dt.bfloat16
F32 = mybir.dt.float32
npbf16 = ml_dtypes.bfloat16

B, S, HID = 4, 4096, 4096
SLOTS, HEADS, BD = 8, 8, 512
HD = BD // HEADS  # 64
N_CORES = 8
GROUPS = N_CORES // B  # head-groups per batch
HPC = HEADS // GROUPS  # heads per core
BDC = HPC * HD  # bottleneck slice per core
MASK_NEG = -30000.0
SCALE = 1.0 / float(np.sqrt(HD))

# test.py can flip this to capture an NTFF profile; harness never touches it.
TRACE = False
TRACE_CORES = None
LAST_RESULT = None

_cache = {}


def _build_module(hid, s, chunk=256):
    """Emit + compile the single-core Bass module (same NEFF on all cores)."""
    nk = hid // 128  # contraction k-tiles
    nrt = s // 128  # 128-row tiles of the sequence
    nch = s // chunk  # row chunks for the K/V projection
    jsub = chunk // 128  # 128-row subtiles per chunk
    nwo = hid // 128  # output tiles of o_proj

    nc = bacc.Bacc("TRN2", target_bir_lowering=False, debug=False, num_devices=N_CORES)

    hsT = nc.dram_tensor("hsT", [hid, s], BF16, kind="ExternalInput").ap()
    wkT = nc.dram_tensor("wkT", [hid, BDC], BF16, kind="ExternalInput").ap()
    wvT = nc.dram_tensor("wvT", [hid, BDC], BF16, kind="ExternalInput").ap()
    wqT = nc.dram_tensor("wqT", [hid, BDC], BF16, kind="ExternalInput").ap()
    woT = nc.dram_tensor("woT", [BDC, hid], BF16, kind="ExternalInput").ap()
    msT = nc.dram_tensor("msT", [hid, SLOTS], BF16, kind="ExternalInput").ap()
    mbT = nc.dram_tensor("mbT", [128, nrt], F32, kind="ExternalInput").ap()
    ypT = nc.dram_tensor("ypT", [hid, SLOTS], F32, kind="ExternalOutput").ap()

    with tile.TileContext(nc) as tc:
        with (
            tc.tile_pool(name="consts", bufs=1) as consts,
            tc.tile_pool(name="hsp", bufs=6) as hsp,
        ):
            # ---- resident weights / tables -------------------------------
            wk_sb = consts.tile([128, nk, BDC], BF16)
            nc.sync.dma_start(
                out=wk_sb, in_=wkT.rearrange("(ko ki) n -> ki ko n", ki=128)
            )
            wv_sb = consts.tile([128, nk, BDC], BF16)
            nc.sync.dma_start(
                out=wv_sb, in_=wvT.rearrange("(ko ki) n -> ki ko n", ki=128)
            )
            wq_sb = consts.tile([128, nk, BDC], BF16)
            nc.sync.dma_start(
                out=wq_sb, in_=wqT.rearrange("(ko ki) n -> ki ko n", ki=128)
            )
            wo_sb = consts.tile([128, BDC // 128, hid], BF16)
            nc.sync.dma_start(
                out=wo_sb, in_=woT.rearrange("(ko ki) n -> ki ko n", ki=128)
            )
            ms_sb = consts.tile([128, nk, SLOTS], BF16)
            nc.sync.dma_start(
                out=ms_sb, in_=msT.rearrange("(ko ki) n -> ki ko n", ki=128)
            )
            mb_sb = consts.tile([128, nrt], F32)
            nc.sync.dma_start(out=mb_sb, in_=mbT)
            ident = consts.tile([128, 128], BF16)
            make_identity(nc, ident)

            # ---- persistent intermediates --------------------------------
            kt_sb = consts.tile([128, BDC // 128, s], BF16)  # K.T  [bd, rows]
            v_sb = consts.tile([128, nrt, HPC, HD + 1], BF16)  # V rows + ones col
            nc.vector.memset(v_sb[:, :, :, HD : HD + 1], 1.0)
            pt_sb = consts.tile([128, nrt, HPC * SLOTS], BF16)  # exp(scores).T
            qt_sb = consts.tile([128, BDC // 128, SLOTS], BF16)  # Q.T [bd, slots]
            ot_sb = consts.tile([128, BDC // 128, SLOTS], BF16)  # o.T [bd, slots]
            yp_sb = consts.tile([128, nwo, SLOTS], F32)
            o_slot = consts.tile([SLOTS, BDC], BF16)  # normalized o [slots, bd]
            recip = consts.tile([SLOTS, HPC], F32)

            # ---- Q projection: QT = WqT.T @ msT --------------------------
            with tc.tile_pool(name="qps", bufs=2, space="PSUM") as qps:
                for m2 in range(BDC // 128):
                    q_ps = qps.tile([128, SLOTS], F32, tag="q")
                    for k in range(nk):
                        nc.tensor.matmul(
                            q_ps,
                            wq_sb[:, k, m2 * 128 : (m2 + 1) * 128],
                            ms_sb[:, k, :],
                            start=(k == 0),
                            stop=(k == nk - 1),
                        )
                    nc.scalar.copy(out=qt_sb[:, m2, :], in_=q_ps)

            # ---- K/V projections, streaming hsT once ---------------------
            with tc.tile_pool(name="kvps", bufs=2, space="PSUM") as kvps:
                for n in range(nch):
                    kt_ps = [
                        kvps.tile([128, chunk], F32, tag=f"kt{m2}", name=f"kt_ps{m2}")
                        for m2 in range(BDC // 128)
                    ]
                    v_ps = [
                        kvps.tile([128, BDC], F32, tag=f"v{j}", name=f"v_ps{j}")
                        for j in range(jsub)
                    ]
                    for k in range(nk):
                        hs_blk = hsp.tile([128, chunk], BF16, tag="hs")
                        nc.sync.dma_start(
                            out=hs_blk,
                            in_=hsT[
                                k * 128 : (k + 1) * 128,
                                n * chunk : (n + 1) * chunk,
                            ],
                        )
                        st, sp = (k == 0), (k == nk - 1)
                        for m2 in range(BDC // 128):
                            nc.tensor.matmul(
                                kt_ps[m2],
                                wk_sb[:, k, m2 * 128 : (m2 + 1) * 128],
                                hs_blk,
                                start=st,
                                stop=sp,
                            )
                        for j in range(jsub):
                            nc.tensor.matmul(
                                v_ps[j],
                                hs_blk[:, j * 128 : (j + 1) * 128],
                                wv_sb[:, k, :],
                                start=st,
                                stop=sp,
                            )
                    for m2 in range(BDC // 128):
                        nc.scalar.copy(
                            out=kt_sb[:, m2, n * chunk : (n + 1) * chunk],
                            in_=kt_ps[m2],
                        )
                    for j in range(jsub):
                        rt = n * jsub + j
                        nc.vector.tensor_copy(
                            out=v_sb[:, rt, :, 0:HD],
                            in_=v_ps[j].rearrange("p (h d) -> p h d", h=HPC),
                        )

            # ---- scores -> exp (all row-tiles) ---------------------------
            oc = consts.tile([SLOTS, HPC, HD + 1], F32)
            with tc.tile_pool(name="aps", bufs=1, space="PSUM") as aps:
                for i in range(nrt):
                    s_ps = aps.tile([128, HPC * SLOTS], F32, tag="s", bufs=2)
                    for h in range(HPC):
                        m2, dof = h // 2, HD * (h % 2)
                        nc.tensor.matmul(
                            s_ps[:, h * SLOTS : (h + 1) * SLOTS],
                            kt_sb[dof : dof + HD, m2, i * 128 : (i + 1) * 128],
                            qt_sb[dof : dof + HD, m2, :],
                            start=True,
                            stop=True,
                        )
                    nc.scalar.activation(
                        out=pt_sb[:, i, :],
                        in_=s_ps,
                        func=mybir.ActivationFunctionType.Exp,
                        bias=mb_sb[:, i : i + 1],
                        scale=1.0,
                    )
            # ---- o = p^T @ V_aug per head --------------------------------
            # Each accumulator gets a full PSUM bank and is drained by ACT:
            # small [8,65] accumulators sharing banks with concurrently
            # DVE-read tiles fault on HW (same-bank PE-W + DVE-R erratum).
            with tc.tile_pool(name="ops", bufs=1, space="PSUM") as ops:
                for h in range(HPC):
                    o_ps = ops.tile([128, 512], F32, tag=f"ob{h}", name=f"o_ps{h}")
                    for i in range(nrt):
                        nc.tensor.matmul(
                            o_ps[0:SLOTS, 0 : HD + 1],
                            pt_sb[:, i, h * SLOTS : (h + 1) * SLOTS],
                            v_sb[:, i, h, :],
                            start=(i == 0),
                            stop=(i == nrt - 1),
                        )
                    nc.scalar.copy(out=oc[:, h, :], in_=o_ps[0:SLOTS, 0 : HD + 1])
            # normalize: o / sum(p), fused via the ones column (SBUF-side)
            for h in range(HPC):
                nc.vector.reciprocal(
                    out=recip[:, h : h + 1], in_=oc[:, h, HD : HD + 1]
                )
                nc.vector.tensor_scalar_mul(
                    out=o_slot[:, h * HD : (h + 1) * HD],
                    in0=oc[:, h, 0:HD],
                    scalar1=recip[:, h : h + 1],
                )

            # ---- transpose o to [bd, slots] ------------------------------
            with tc.tile_pool(name="tps", bufs=2, space="PSUM") as tps:
                for j in range(BDC // 128):
                    t_ps = tps.tile([128, SLOTS], BF16, tag="t")
                    nc.tensor.transpose(
                        t_ps,
                        o_slot[:, j * 128 : (j + 1) * 128],
                        ident[:SLOTS, :SLOTS],
                    )
                    nc.scalar.copy(out=ot_sb[:, j, :], in_=t_ps)

            # ---- partial o_proj: ypT = WoT.T @ OT ------------------------
            with tc.tile_pool(name="yps", bufs=4, space="PSUM") as yps:
                for m in range(nwo):
                    y_ps = yps.tile([128, SLOTS], F32, tag="y")
                    for k2 in range(BDC // 128):
                        nc.tensor.matmul(
                            y_ps,
                            wo_sb[:, k2, m * 128 : (m + 1) * 128],
                            ot_sb[:, k2, :],
                            start=(k2 == 0),
                            stop=(k2 == BDC // 128 - 1),
                        )
                    nc.vector.tensor_copy(out=yp_sb[:, m, :], in_=y_ps)
                nc.sync.dma_start(
                    out=ypT.rearrange("(mo mi) n -> mi mo n", mi=128), in_=yp_sb
                )

    nc.compile()
    return nc


def _get_module():
    key = (HID, S)
    if key not in _cache:
        _cache[key] = _build_module(HID, S)
    return _cache[key]


def _prep_in_maps(hs, mask, ms, Wq, Wk, Wv, Wo):
    """Shard the full inputs into 8 per-core input maps (host-side)."""
    hsT = [np.ascontiguousarray(hs[b].T.astype(npbf16)) for b in range(B)]
    mb = [
        np.ascontiguousarray(
            np.where(mask[b] == 0, np.float32(MASK_NEG), np.float32(0.0))
            .astype(np.float32)
            .reshape(S // 128, 128)
            .T
        )
        for b in range(B)
    ]
    msT = np.ascontiguousarray((ms.T * SCALE).astype(npbf16))
    WqT = Wq.T.astype(npbf16)  # [HID, BD]
    WkT = Wk.T.astype(npbf16)
    WvT = Wv.T.astype(npbf16)
    WoT = Wo.T.astype(npbf16)  # [BD, HID]

    in_maps = []
    for c in range(N_CORES):
        b, g = c // GROUPS, c % GROUPS
        sl = slice(g * BDC, (g + 1) * BDC)
        in_maps.append(
            {
                "hsT": hsT[b],
                "wkT": np.ascontiguousarray(WkT[:, sl]),
                "wvT": np.ascontiguousarray(WvT[:, sl]),
                "wqT": np.ascontiguousarray(WqT[:, sl]),
                "woT": np.ascontiguousarray(WoT[sl, :]),
                "msT": msT,
                "mbT": mb[b],
            }
        )
    return in_maps


def time_device(inputs_np, reps=8):
    """Dev-only helper (not used by grading): time repeated NEFF executions
    with inputs resident on device. Mirrors bass2jax.run_bass_via_pjrt's
    multi-core path; each wall time includes one axon execute round-trip."""
    import time

    import jax
    from jax.experimental.shard_map import shard_map
    from jax.sharding import Mesh, NamedSharding, PartitionSpec

    import concourse.mybir as mybir_
    from concourse import bass2jax

    nc = _get_module()
    in_maps = _prep_in_maps(
        np.asarray(inputs_np["hidden_states"], np.float32),
        np.asarray(inputs_np["attention_mask"]),
        np.asarray(inputs_np["memory_slots"], np.float32),
        np.asarray(inputs_np["Wq"], np.float32),
        np.asarray(inputs_np["Wk"], np.float32),
        np.asarray(inputs_np["Wv"], np.float32),
        np.asarray(inputs_np["Wo"], np.float32),
    )
    bass2jax.install_neuronx_cc_hook()

    in_names, out_names, out_avals, zero_outs = [], [], [], []
    has_partition = False
    for alloc in nc.m.functions[0].allocations:
        if not isinstance(alloc, mybir_.MemoryLocationSet):
            continue
        name = alloc.memorylocations[0].name
        if alloc.kind == "ExternalInput":
            if name == "partition_id":
                has_partition = True
                continue
            in_names.append(name)
        elif alloc.kind == "ExternalOutput":
            out_names.append(name)
            shape = tuple(alloc.tensor_shape)
            dtype = mybir_.dt.np(alloc.dtype)
            out_avals.append(jax.core.ShapedArray(shape, dtype))
            zero_outs.append(np.zeros(shape, dtype))
    n_params = len(in_names)
    n_outs = len(out_avals)
    all_names = in_names + (["partition_id"] if has_partition else []) + out_names

    def _body(*args):
        operands = list(args[:n_params])
        if has_partition:
            operands.append(bass2jax.partition_id_tensor())
        operands += list(args[n_params:])
        outs = bass2jax._bass_exec_p.bind(
            *operands,
            out_avals=tuple(out_avals),
            in_names=tuple(all_names),
            out_names=tuple(out_names),
            lowering_input_output_aliases=(),
            sim_require_finite=True,
            sim_require_nnan=True,
            nc=nc,
        )
        return tuple(outs)

    devices = jax.devices()[:N_CORES]
    mesh = Mesh(np.asarray(devices), ("core",))
    spec = PartitionSpec("core")
    sharded = jax.jit(
        shard_map(
            _body,
            mesh=mesh,
            in_specs=(spec,) * (n_params + n_outs),
            out_specs=(spec,) * n_outs,
            check_rep=False,
        ),
        donate_argnums=tuple(range(n_params, n_params + n_outs)),
        keep_unused=True,
    )
    concat_in = [
        np.concatenate([np.asarray(in_maps[c][nm]) for c in range(N_CORES)], axis=0)
        for nm in in_names
    ]
    sh = NamedSharding(mesh, spec)
    dev_in = [jax.device_put(a, sh) for a in concat_in]
    jax.block_until_ready(dev_in)

    times = []
    for _ in range(reps):
        zeros = [np.zeros((N_CORES * z.shape[0], *z.shape[1:]), z.dtype)
                 for z in zero_outs]
        dz = [jax.device_put(z, sh) for z in zeros]
        jax.block_until_ready(dz)
        t0 = time.perf_counter()
        out = sharded(*dev_in, *dz)
        jax.block_until_ready(out)
        times.append(time.perf_counter() - t0)
    return times


def kernel(hidden_states, attention_mask, memory_slots, Wq, Wk, Wv, Wo):
    global LAST_RESULT
    hs = np.asarray(hidden_states, dtype=np.float32)
    mask = np.asarray(attention_mask)
    ms = np.asarray(memory_slots, dtype=np.float32)
    Wq = np.asarray(Wq, dtype=np.float32)
    Wk = np.asarray(Wk, dtype=np.float32)
    Wv = np.asarray(Wv, dtype=np.float32)
    Wo = np.asarray(Wo, dtype=np.float32)

    nc = _get_module()
    in_maps = _prep_in_maps(hs, mask, ms, Wq, Wk, Wv, Wo)

    kwargs = {}
    if TRACE:
        kwargs = {"trace": True}
        if TRACE_CORES is not None:
            kwargs["trace_cores"] = TRACE_CORES
    res = run_bass_kernel_spmd(nc, in_maps, core_ids=list(range(N_CORES)), **kwargs)
    LAST_RESULT = res

    yp = [r["ypT"] for r in res.results]  # each [HID, SLOTS] f32
    y = np.stack(
        [(yp[GROUPS * b] + yp[GROUPS * b + 1]).T for b in range(B)], axis=0
    )
    return np.ascontiguousarray(y.astype(np.float32))



# revision 2
# speedup vs baseline: 1.1739x; 1.1739x over previous
"""Trainium2 Bass kernel: memory-slot cross-attention (nn_LocalConstructorMulti).

Reference computation (per batch b):
    Q  = memory_slots @ Wq.T                      [slots, BD]    (shared over b)
    K  = hs_b @ Wk.T                              [S, BD]
    V  = hs_b @ Wv.T                              [S, BD]
    s  = (Q_h . K_h) / sqrt(HD)  + mask           [heads, slots, S]
    p  = softmax(s, axis=S)
    o  = p @ V_h                                  [heads, slots, HD]
    y  = concat_h(o) @ Wo.T                       [slots, HID]

Sharding: 8 cores = 4 batches x 2 head-groups (4 heads / 256 bottleneck dims
each).  Each core sees the full (transposed, bf16) hidden states of its batch
and a 256-wide slice of Wq/Wk/Wv/Wo, computes the full softmax locally over
its heads, and produces a partial y (contribution of its 4 heads).  The host
sums the two partials per batch -- o_proj is linear in the per-head outputs,
so no flash-softmax combine is needed.

Device layout notes:
  - hs arrives pre-transposed as hsT [HID, S] so the contraction dim (HID) is
    on partitions for both the K-path (hs as moving operand) and the V-path
    (hs as stationary operand).  No on-device transposes of the big tensor.
  - K is built as KT [256, S] (bd on partitions) for the Q.K matmuls;
    V is built as V [S, 256] (rows on partitions) for the p@V matmuls.
  - scores are built transposed, sT [rows, heads*slots], so the additive
    sequence mask is a per-partition bias fused into the Exp activation.
  - softmax denominator comes for free: V tiles carry an extra ones column,
    so o_psum[:, 64] accumulates sum(p) and normalization is a per-partition
    tensor_scalar multiply.
"""

import sys

if "/opt/trn_rl_repo" not in sys.path:
    sys.path.insert(0, "/opt/trn_rl_repo")

import ml_dtypes
import numpy as np

import concourse.bass as bass  # noqa: F401  (AP helpers)
import concourse.mybir as mybir
import concourse.tile as tile
from concourse import bacc
from concourse.bass_utils import run_bass_kernel_spmd
from concourse.masks import make_identity

BF16 = mybir.dt.bfloat16
F32 = mybir.dt.float32
npbf16 = ml_dtypes.bfloat16

B, S, HID = 4, 4096, 4096
SLOTS, HEADS, BD = 8, 8, 512
HD = BD // HEADS  # 64
N_CORES = 8
GROUPS = N_CORES // B  # head-groups per batch
HPC = HEADS // GROUPS  # heads per core
BDC = HPC * HD  # bottleneck slice per core
MASK_NEG = -30000.0
SCALE = 1.0 / float(np.sqrt(HD))

# test.py can flip this to capture an NTFF profile; harness never touches it.
TRACE = False
TRACE_CORES = None
LAST_RESULT = None

_cache = {}


def _build_module(hid, s, chunk=256):
    """Emit + compile the single-core Bass module (same NEFF on all cores)."""
    nk = hid // 128  # contraction k-tiles
    nrt = s // 128  # 128-row tiles of the sequence
    nch = s // chunk  # row chunks for the K/V projection
    jsub = chunk // 128  # 128-row subtiles per chunk
    nwo = hid // 128  # output tiles of o_proj

    nc = bacc.Bacc("TRN2", target_bir_lowering=False, debug=False, num_devices=N_CORES)

    hsT = nc.dram_tensor("hsT", [hid, s], BF16, kind="ExternalInput").ap()
    wkT = nc.dram_tensor("wkT", [hid, BDC], BF16, kind="ExternalInput").ap()
    wvT = nc.dram_tensor("wvT", [hid, BDC], BF16, kind="ExternalInput").ap()
    wqT = nc.dram_tensor("wqT", [hid, BDC], BF16, kind="ExternalInput").ap()
    woT = nc.dram_tensor("woT", [BDC, hid], BF16, kind="ExternalInput").ap()
    msT = nc.dram_tensor("msT", [hid, SLOTS], BF16, kind="ExternalInput").ap()
    mbT = nc.dram_tensor("mbT", [128, nrt], F32, kind="ExternalInput").ap()
    ypT = nc.dram_tensor("ypT", [hid, SLOTS], F32, kind="ExternalOutput").ap()

    with tile.TileContext(nc) as tc:
        with (
            tc.tile_pool(name="consts", bufs=1) as consts,
            tc.tile_pool(name="hsp", bufs=6) as hsp,
        ):
            # ---- resident weights / tables -------------------------------
            wk_sb = consts.tile([128, nk, BDC], BF16)
            nc.sync.dma_start(
                out=wk_sb, in_=wkT.rearrange("(ko ki) n -> ki ko n", ki=128)
            )
            wv_sb = consts.tile([128, nk, BDC], BF16)
            nc.sync.dma_start(
                out=wv_sb, in_=wvT.rearrange("(ko ki) n -> ki ko n", ki=128)
            )
            wq_sb = consts.tile([128, nk, BDC], BF16)
            nc.sync.dma_start(
                out=wq_sb, in_=wqT.rearrange("(ko ki) n -> ki ko n", ki=128)
            )
            wo_sb = consts.tile([128, BDC // 128, hid], BF16)
            nc.sync.dma_start(
                out=wo_sb, in_=woT.rearrange("(ko ki) n -> ki ko n", ki=128)
            )
            ms_sb = consts.tile([128, nk, SLOTS], BF16)
            nc.sync.dma_start(
                out=ms_sb, in_=msT.rearrange("(ko ki) n -> ki ko n", ki=128)
            )
            mb_sb = consts.tile([128, nrt], F32)
            nc.sync.dma_start(out=mb_sb, in_=mbT)
            ident = consts.tile([128, 128], BF16)
            make_identity(nc, ident)

            # ---- persistent intermediates --------------------------------
            kt_sb = consts.tile([128, BDC // 128, s], BF16)  # K.T  [bd, rows]
            v_sb = consts.tile([128, nrt, HPC, HD + 1], BF16)  # V rows + ones col
            nc.vector.memset(v_sb[:, :, :, HD : HD + 1], 1.0)
            pt_sb = consts.tile([128, nrt, HPC * SLOTS], BF16)  # exp(scores).T
            qt_sb = consts.tile([128, BDC // 128, SLOTS], BF16)  # Q.T [bd, slots]
            ot_sb = consts.tile([128, BDC // 128, SLOTS], BF16)  # o.T [bd, slots]
            yp_sb = consts.tile([128, nwo, SLOTS], F32)
            o_slot = consts.tile([SLOTS, BDC], BF16)  # normalized o [slots, bd]
            recip = consts.tile([SLOTS, HPC], F32)

            # ---- Q projection: QT = WqT.T @ msT --------------------------
            with tc.tile_pool(name="qps", bufs=2, space="PSUM") as qps:
                for m2 in range(BDC // 128):
                    q_ps = qps.tile([128, SLOTS], F32, tag="q")
                    for k in range(nk):
                        nc.tensor.matmul(
                            q_ps,
                            wq_sb[:, k, m2 * 128 : (m2 + 1) * 128],
                            ms_sb[:, k, :],
                            start=(k == 0),
                            stop=(k == nk - 1),
                        )
                    nc.scalar.copy(out=qt_sb[:, m2, :], in_=q_ps)

            # ---- K/V projections, streaming hsT once ---------------------
            with tc.tile_pool(name="kvps", bufs=2, space="PSUM") as kvps:
                for n in range(nch):
                    kt_ps = [
                        kvps.tile([128, chunk], F32, tag=f"kt{m2}", name=f"kt_ps{m2}")
                        for m2 in range(BDC // 128)
                    ]
                    v_ps = [
                        kvps.tile([128, BDC], F32, tag=f"v{j}", name=f"v_ps{j}")
                        for j in range(jsub)
                    ]
                    for k in range(nk):
                        hs_blk = hsp.tile([128, chunk], BF16, tag="hs")
                        nc.sync.dma_start(
                            out=hs_blk,
                            in_=hsT[
                                k * 128 : (k + 1) * 128,
                                n * chunk : (n + 1) * chunk,
                            ],
                        )
                        st, sp = (k == 0), (k == nk - 1)
                        for m2 in range(BDC // 128):
                            nc.tensor.matmul(
                                kt_ps[m2],
                                wk_sb[:, k, m2 * 128 : (m2 + 1) * 128],
                                hs_blk,
                                start=st,
                                stop=sp,
                            )
                        for j in range(jsub):
                            nc.tensor.matmul(
                                v_ps[j],
                                hs_blk[:, j * 128 : (j + 1) * 128],
                                wv_sb[:, k, :],
                                start=st,
                                stop=sp,
                            )
                    for m2 in range(BDC // 128):
                        nc.scalar.copy(
                            out=kt_sb[:, m2, n * chunk : (n + 1) * chunk],
                            in_=kt_ps[m2],
                        )
                    for j in range(jsub):
                        rt = n * jsub + j
                        nc.vector.tensor_copy(
                            out=v_sb[:, rt, :, 0:HD],
                            in_=v_ps[j].rearrange("p (h d) -> p h d", h=HPC),
                        )

            # ---- scores -> exp (all row-tiles) ---------------------------
            oc = consts.tile([SLOTS, HPC, HD + 1], F32)
            with tc.tile_pool(name="aps", bufs=1, space="PSUM") as aps:
                for i in range(nrt):
                    s_ps = aps.tile([128, HPC * SLOTS], F32, tag="s", bufs=2)
                    for h in range(HPC):
                        m2, dof = h // 2, HD * (h % 2)
                        nc.tensor.matmul(
                            s_ps[:, h * SLOTS : (h + 1) * SLOTS],
                            kt_sb[dof : dof + HD, m2, i * 128 : (i + 1) * 128],
                            qt_sb[dof : dof + HD, m2, :],
                            start=True,
                            stop=True,
                        )
                    nc.scalar.activation(
                        out=pt_sb[:, i, :],
                        in_=s_ps,
                        func=mybir.ActivationFunctionType.Exp,
                        bias=mb_sb[:, i : i + 1],
                        scale=1.0,
                    )
            # ---- o = p^T @ V_aug per head --------------------------------
            # Each accumulator gets a full PSUM bank and is drained by ACT:
            # small [8,65] accumulators sharing banks with concurrently
            # DVE-read tiles fault on HW (same-bank PE-W + DVE-R erratum).
            with tc.tile_pool(name="ops", bufs=1, space="PSUM") as ops:
                for h in range(HPC):
                    o_ps = ops.tile([128, 512], F32, tag=f"ob{h}", name=f"o_ps{h}")
                    for i in range(nrt):
                        nc.tensor.matmul(
                            o_ps[0:SLOTS, 0 : HD + 1],
                            pt_sb[:, i, h * SLOTS : (h + 1) * SLOTS],
                            v_sb[:, i, h, :],
                            start=(i == 0),
                            stop=(i == nrt - 1),
                        )
                    nc.scalar.copy(out=oc[:, h, :], in_=o_ps[0:SLOTS, 0 : HD + 1])
            # normalize: o / sum(p), fused via the ones column (SBUF-side)
            for h in range(HPC):
                nc.vector.reciprocal(
                    out=recip[:, h : h + 1], in_=oc[:, h, HD : HD + 1]
                )
                nc.vector.tensor_scalar_mul(
                    out=o_slot[:, h * HD : (h + 1) * HD],
                    in0=oc[:, h, 0:HD],
                    scalar1=recip[:, h : h + 1],
                )

            # ---- transpose o to [bd, slots] ------------------------------
            with tc.tile_pool(name="tps", bufs=2, space="PSUM") as tps:
                for j in range(BDC // 128):
                    t_ps = tps.tile([128, SLOTS], BF16, tag="t")
                    nc.tensor.transpose(
                        t_ps,
                        o_slot[:, j * 128 : (j + 1) * 128],
                        ident[:SLOTS, :SLOTS],
                    )
                    nc.scalar.copy(out=ot_sb[:, j, :], in_=t_ps)

            # ---- partial o_proj: ypT = WoT.T @ OT ------------------------
            with tc.tile_pool(name="yps", bufs=4, space="PSUM") as yps:
                for m in range(nwo):
                    y_ps = yps.tile([128, SLOTS], F32, tag="y")
                    for k2 in range(BDC // 128):
                        nc.tensor.matmul(
                            y_ps,
                            wo_sb[:, k2, m * 128 : (m + 1) * 128],
                            ot_sb[:, k2, :],
                            start=(k2 == 0),
                            stop=(k2 == BDC // 128 - 1),
                        )
                    nc.vector.tensor_copy(out=yp_sb[:, m, :], in_=y_ps)
                nc.sync.dma_start(
                    out=ypT.rearrange("(mo mi) n -> mi mo n", mi=128), in_=yp_sb
                )

    nc.compile()
    return nc


def _get_module():
    key = (HID, S)
    if key not in _cache:
        _cache[key] = _build_module(HID, S)
    return _cache[key]


def _prep_in_maps(hs, mask, ms, Wq, Wk, Wv, Wo):
    """Shard the full inputs into 8 per-core input maps (host-side)."""
    hsT = [np.ascontiguousarray(hs[b].T.astype(npbf16)) for b in range(B)]
    mb = [
        np.ascontiguousarray(
            np.where(mask[b] == 0, np.float32(MASK_NEG), np.float32(0.0))
            .astype(np.float32)
            .reshape(S // 128, 128)
            .T
        )
        for b in range(B)
    ]
    msT = np.ascontiguousarray((ms.T * SCALE).astype(npbf16))
    WqT = Wq.T.astype(npbf16)  # [HID, BD]
    WkT = Wk.T.astype(npbf16)
    WvT = Wv.T.astype(npbf16)
    WoT = Wo.T.astype(npbf16)  # [BD, HID]

    in_maps = []
    for c in range(N_CORES):
        b, g = c // GROUPS, c % GROUPS
        sl = slice(g * BDC, (g + 1) * BDC)
        in_maps.append(
            {
                "hsT": hsT[b],
                "wkT": np.ascontiguousarray(WkT[:, sl]),
                "wvT": np.ascontiguousarray(WvT[:, sl]),
                "wqT": np.ascontiguousarray(WqT[:, sl]),
                "woT": np.ascontiguousarray(WoT[sl, :]),
                "msT": msT,
                "mbT": mb[b],
            }
        )
    return in_maps


def time_device(inputs_np, reps=8):
    """Dev-only helper (not used by grading): time repeated NEFF executions
    with inputs resident on device. Mirrors bass2jax.run_bass_via_pjrt's
    multi-core path; each wall time includes one axon execute round-trip."""
    import time

    import jax
    from jax.experimental.shard_map import shard_map
    from jax.sharding import Mesh, NamedSharding, PartitionSpec

    import concourse.mybir as mybir_
    from concourse import bass2jax

    nc = _get_module()
    in_maps = _prep_in_maps(
        np.asarray(inputs_np["hidden_states"], np.float32),
        np.asarray(inputs_np["attention_mask"]),
        np.asarray(inputs_np["memory_slots"], np.float32),
        np.asarray(inputs_np["Wq"], np.float32),
        np.asarray(inputs_np["Wk"], np.float32),
        np.asarray(inputs_np["Wv"], np.float32),
        np.asarray(inputs_np["Wo"], np.float32),
    )
    bass2jax.install_neuronx_cc_hook()

    in_names, out_names, out_avals, zero_outs = [], [], [], []
    has_partition = False
    for alloc in nc.m.functions[0].allocations:
        if not isinstance(alloc, mybir_.MemoryLocationSet):
            continue
        name = alloc.memorylocations[0].name
        if alloc.kind == "ExternalInput":
            if name == "partition_id":
                has_partition = True
                continue
            in_names.append(name)
        elif alloc.kind == "ExternalOutput":
            out_names.append(name)
            shape = tuple(alloc.tensor_shape)
            dtype = mybir_.dt.np(alloc.dtype)
            out_avals.append(jax.core.ShapedArray(shape, dtype))
            zero_outs.append(np.zeros(shape, dtype))
    n_params = len(in_names)
    n_outs = len(out_avals)
    # Operand order must match run_bass_via_pjrt: inputs, donated output
    # zeros, then partition-id LAST (neuronx_cc_hook checks operands[:-1]
    # are jit parameters 0..N-1).
    all_names = in_names + out_names + (["partition_id"] if has_partition else [])

    def _body(*args):
        operands = list(args)
        if has_partition:
            operands.append(bass2jax.partition_id_tensor())
        outs = bass2jax._bass_exec_p.bind(
            *operands,
            out_avals=tuple(out_avals),
            in_names=tuple(all_names),
            out_names=tuple(out_names),
            lowering_input_output_aliases=(),
            sim_require_finite=True,
            sim_require_nnan=True,
            nc=nc,
        )
        return tuple(outs)

    devices = jax.devices()[:N_CORES]
    mesh = Mesh(np.asarray(devices), ("core",))
    spec = PartitionSpec("core")
    sharded = jax.jit(
        shard_map(
            _body,
            mesh=mesh,
            in_specs=(spec,) * (n_params + n_outs),
            out_specs=(spec,) * n_outs,
            check_rep=False,
        ),
        donate_argnums=tuple(range(n_params, n_params + n_outs)),
        keep_unused=True,
    )
    concat_in = [
        np.concatenate([np.asarray(in_maps[c][nm]) for c in range(N_CORES)], axis=0)
        for nm in in_names
    ]
    sh = NamedSharding(mesh, spec)
    dev_in = [jax.device_put(a, sh) for a in concat_in]
    jax.block_until_ready(dev_in)

    times = []
    for _ in range(reps):
        zeros = [np.zeros((N_CORES * z.shape[0], *z.shape[1:]), z.dtype)
                 for z in zero_outs]
        dz = [jax.device_put(z, sh) for z in zeros]
        jax.block_until_ready(dz)
        t0 = time.perf_counter()
        out = sharded(*dev_in, *dz)
        jax.block_until_ready(out)
        times.append(time.perf_counter() - t0)
    return times


def kernel(hidden_states, attention_mask, memory_slots, Wq, Wk, Wv, Wo):
    global LAST_RESULT
    hs = np.asarray(hidden_states, dtype=np.float32)
    mask = np.asarray(attention_mask)
    ms = np.asarray(memory_slots, dtype=np.float32)
    Wq = np.asarray(Wq, dtype=np.float32)
    Wk = np.asarray(Wk, dtype=np.float32)
    Wv = np.asarray(Wv, dtype=np.float32)
    Wo = np.asarray(Wo, dtype=np.float32)

    nc = _get_module()
    in_maps = _prep_in_maps(hs, mask, ms, Wq, Wk, Wv, Wo)

    kwargs = {}
    if TRACE:
        kwargs = {"trace": True}
        if TRACE_CORES is not None:
            kwargs["trace_cores"] = TRACE_CORES
    res = run_bass_kernel_spmd(nc, in_maps, core_ids=list(range(N_CORES)), **kwargs)
    LAST_RESULT = res

    yp = [r["ypT"] for r in res.results]  # each [HID, SLOTS] f32
    y = np.stack(
        [(yp[GROUPS * b] + yp[GROUPS * b + 1]).T for b in range(B)], axis=0
    )
    return np.ascontiguousarray(y.astype(np.float32))

